# revision 5
# baseline (speedup 1.0000x reference)
"""Trainium2 Bass kernel for nn_CosineDistance (retrieval maxsim).

Reference computation:
    pano_n = l2norm(pano [64,64,128]);  sat_n = l2norm(sat [256,64,128])
    sim[a,b,i,j] = pano_n[a,i,:] . sat_n[b,j,:]
    out[a,b] = sim.max(axis=j).sum(axis=i)           -> [64, 256] fp32

Sharding: sat (b) axis split across 8 cores, 32 sats each. Each core
computes the full [64, 32] slice of the output; host concatenates.

Device algorithm per core (bf16 matmuls, fp32 PSUM):
  - prep: satN [2048,128] f32 token-major -> sum-of-squares (ACT square +
    DVE reduce) -> sqrt (ACT) -> reciprocal (DVE) -> per-token scale
    (GpSimd tensor_scalar -> bf16) -> bf16 DMA-transpose roundtrip through
    DRAM -> satT [128d, 2048tok].  panoT arrives pre-transposed from host.
  - pano norms: GpSimd squares panoT (frees ACT/DVE); 32 matmuls vs ones
    -> |p|^2 in PSUM -> sqrt -> reciprocal -> pscale, folded with the
    block-ones pattern into bf16 stage-2 matmul weights (fold).
  - main loop, 32 pano-pairs x [128, 2048] fp32 sim tiles (4 PSUM banks,
    2 in flight). Two drain paths balanced so DVE and ACT both stay busy:
      direct (x~1/4): one DVE reduce_max [128,(32s,64j)] -> maxsb bf16.
      tree: one ACT copy [128,2048] PSUM->SBUF bf16, then a batched DVE
      pairwise-max tree (bf16 2x) over TREE_RUN pairs -> maxsb.
    The PSUM drain is port-limited (1 elem/cycle/lane on both DVE and ACT
    for fp32 PSUM, tensor_reduce is 1x-only, GpSimd/DMA cannot touch
    PSUM), so this split is the roofline for the j-max.
  - stage2 (two halves): per-pair mini-matmul with fold as stationary
    sums the 64 pano tokens and applies 1/|p| in one shot -> [2,512]
    PSUM -> ACT copy -> out_sb -> DMA out [64, 32].
"""

import numpy as np

N_CORES = 8
A, I, D = 64, 64, 128          # panos, pano tokens, dim
B, J = 256, 64                 # sats, sat tokens
B_SH = B // N_CORES            # 32 sats per core
TOK_SAT = B_SH * J             # 2048 sat tokens per core
TOK_PANO = A * I               # 4096 pano tokens
PAIRS = A // 2                 # 32 pano pairs

# Schedule: LEAD_D directs first (ACT busy with prep), then runs of
# TREE_RUN tree-pairs (one batch each) separated by single directs.
LEAD_D = 2
TREE_RUN = 4
N_DIRECT = 8
GPS_SQ_PANO = True             # pano squaring on GpSimd (else ACT)
# Allocate the pano-norm PSUM tile after this many pairs: late enough that
# the PE (paced by the drains, ~2 pairs ahead) reaches the norm matmuls only
# after the GpSimd pano-square has finished, early enough for stage2 at 16.
NORM_AFTER = 10

_CACHE = {}


def _schedule():
    kinds = ["D"] * LEAD_D
    directs_left = N_DIRECT - LEAD_D
    while len(kinds) < PAIRS:
        run = min(TREE_RUN, PAIRS - len(kinds) - directs_left)
        kinds += ["T"] * run
        if directs_left > 0 and len(kinds) < PAIRS:
            kinds.append("D")
            directs_left -= 1
    return [(p, k == "D") for p, k in enumerate(kinds)]


def _build_nc(repeat=1):
    import concourse.bass as bass
    import concourse.bacc as bacc
    import concourse.tile as tile
    from concourse import mybir

    f32 = mybir.dt.float32
    bf16 = mybir.dt.bfloat16

    nc = bacc.Bacc("TRN2", target_bir_lowering=False, debug=False)
    satN_d = nc.declare_dram_parameter("satN", [TOK_SAT, D], f32, isOutput=False)
    panoT_d = nc.declare_dram_parameter("panoT", [D, TOK_PANO], bf16, isOutput=False)
    consts_d = nc.declare_dram_parameter("consts", [128, 3], bf16, isOutput=False)
    out_d = nc.declare_dram_parameter("out", [A, B_SH], f32, isOutput=True)

    with tile.TileContext(nc) as tc:
     for _rep in range(repeat):
        with (
            tc.tile_pool(name="persist", bufs=1) as persist,
        ):
            NT = TOK_SAT // 128          # 16 sat tiles of 128 tokens
            NH = NT // 2                 # tiles per half-chunk

            satT = persist.tile([128, TOK_SAT], bf16)
            panoT_sb = persist.tile([128, TOK_PANO], bf16)
            consts_sb = persist.tile([128, 4], bf16)
            maxsb = persist.tile([128, PAIRS * B_SH], bf16)   # [128, 1024]
            pscale = persist.tile([128, PAIRS], f32)
            fold = persist.tile([128, 2 * PAIRS], bf16)       # stage2 weights
            out_sb = persist.tile([2, PAIRS * B_SH], f32)
            sq_pano = persist.tile([128, TOK_PANO], bf16)

            satN_r = satN_d[:].rearrange("(t p) d -> p t d", p=128)

            # pano DMA first: gpsimd square is the longest prep chain
            nc.sync.dma_start(out=panoT_sb, in_=panoT_d[:])
            nc.sync.dma_start(out=consts_sb[:, 0:3], in_=consts_d[:])
            ones_blk = consts_sb[:, 0:2]
            ones1 = consts_sb[:, 2:3]
            if GPS_SQ_PANO:
                nc.gpsimd.tensor_tensor(
                    out=sq_pano, in0=panoT_sb, in1=panoT_sb,
                    op=mybir.AluOpType.mult,
                )
            else:
                nc.scalar.square(sq_pano, panoT_sb)

            with (
                tc.tile_pool(name="prep", bufs=1) as prep,
            ):
                satN_sb = prep.tile([128, NT, D], f32)
                sq_sat = prep.tile([128, NT, D], f32)
                nrm_sat = prep.tile([128, NT], f32)
                sscale = prep.tile([128, NT], f32)
                satn_bf = prep.tile([128, NT, D], bf16)

                # sat DMA + normalize, in 2 pipelined half-chunks
                for h in range(2):
                    tl = slice(NH * h, NH * (h + 1))
                    nc.sync.dma_start(out=satN_sb[:, tl, :], in_=satN_r[:, tl, :])
                    nc.scalar.square(sq_sat[:, tl, :], satN_sb[:, tl, :])
                    nc.vector.reduce_sum(
                        out=nrm_sat[:, tl],
                        in_=sq_sat[:, tl, :],
                        axis=mybir.AxisListType.X,
                    )
                    nc.scalar.sqrt(nrm_sat[:, tl], nrm_sat[:, tl])
                    nc.vector.reciprocal(sscale[:, tl], nrm_sat[:, tl])
                    for t in range(NH * h, NH * (h + 1)):
                        nc.gpsimd.tensor_scalar_mul(
                            satn_bf[:, t, :], satN_sb[:, t, :], sscale[:, t : t + 1]
                        )

                # satT via bf16 DMA-transpose roundtrip through DRAM — frees
                # the PE (no identity-matmul transposes) and ScalarE (no
                # PSUM->SBUF copies); DMA engines are otherwise idle.
                satn_dram = nc.dram_tensor(f"satn_dram_{_rep}", [TOK_SAT, D], bf16)
                satn_dram_r = satn_dram[:].rearrange("(t p) d -> p t d", p=128)
                for h in range(2):
                    tl = slice(NH * h, NH * (h + 1))
                    nc.sync.dma_start(out=satn_dram_r[:, tl, :], in_=satn_bf[:, tl, :])
                    nc.sync.dma_start_transpose(
                        out=satT[:, 1024 * h : 1024 * (h + 1)],
                        in_=satn_dram[1024 * h : 1024 * (h + 1), :],
                    )

            # ---------------- main loop ------------------------------
            with (
                tc.tile_pool(name="treep", bufs=2) as treep,
                tc.tile_pool(name="sim_psum", bufs=2, space="PSUM") as spsum,
            ):
                def mm_pair(ps, p):
                    for g in range(4):
                        nc.tensor.matmul(
                            ps[:, 512 * g : 512 * (g + 1)],
                            panoT_sb[:, 128 * p : 128 * (p + 1)],
                            satT[:, 512 * g : 512 * (g + 1)],
                            start=True,
                            stop=True,
                        )

                def pano_norms():
                    # |p|^2 via 32 matmuls of squared panoT against ones
                    ps_n = spsum.tile([128, 2048], f32, tag="sim", name="ps_n")
                    for q in range(PAIRS):
                        nc.tensor.matmul(
                            ps_n[:, q : q + 1],
                            sq_pano[:, 128 * q : 128 * (q + 1)],
                            ones1,
                            start=True,
                            stop=True,
                        )
                    nrm_pano = treep.tile([128, PAIRS], f32, tag="nrmp")
                    nc.scalar.sqrt(nrm_pano, ps_n[:, 0:PAIRS])
                    nc.vector.reciprocal(pscale, nrm_pano)
                    # fold[:, 2q+r] = pscale[:, q] * ones_blk[:, r]
                    fold_v = fold[:].rearrange("p (q r) -> p q r", r=2)
                    for r in range(2):
                        nc.vector.tensor_tensor(
                            out=fold_v[:, :, r],
                            in0=pscale,
                            in1=ones_blk[:, r : r + 1].to_broadcast([128, PAIRS]),
                            op=mybir.AluOpType.mult,
                        )

                def flush_tree(queue):
                    if not queue:
                        return
                    nb = len(queue)
                    p0 = queue[0][0]
                    simcp = queue[0][1]
                    # pairwise-max tree over j (innermost 64), DVE bf16 2x
                    w = 32
                    src = simcp[:, : nb * 2048].rearrange("p (q j) -> p q j", j=J)
                    while w >= 1:
                        if w > 1:
                            dst_t = treep.tile(
                                [128, TREE_RUN * 32 * w], bf16, tag=f"tree{w}",
                                name=f"tree{w}_{p0}",
                            )
                            dst = dst_t[:, : nb * 32 * w].rearrange(
                                "p (q j) -> p q j", j=w
                            )
                        else:
                            dst = maxsb[:, B_SH * p0 : B_SH * (p0 + nb), None]
                        nc.vector.tensor_tensor(
                            out=dst,
                            in0=src[:, :, 0:w],
                            in1=src[:, :, w : 2 * w],
                            op=mybir.AluOpType.max,
                        )
                        if w > 1:
                            src = dst
                        w //= 2
                    queue.clear()

                def stage2_half(h):
                    # fold (bf16, pscale*blockones) as stationary: sums the
                    # 64 pano tokens of each pair and applies 1/|p|
                    ps2 = spsum.tile([128, 2048], f32, tag="sim", name=f"ps2_{h}")
                    for u in range(16):
                        q = 16 * h + u
                        nc.tensor.matmul(
                            ps2[0:2, 32 * u : 32 * (u + 1)],
                            fold[:, 2 * q : 2 * (q + 1)],
                            maxsb[:, B_SH * q : B_SH * (q + 1)],
                            start=True,
                            stop=True,
                        )
                    nc.scalar.copy(out_sb[:, 512 * h : 512 * (h + 1)], ps2[0:2, 0:512])

                queue = []
                done = 0
                for p, is_direct in _schedule():
                    ps = spsum.tile([128, 2048], f32, tag="sim", name=f"ps_{p}")
                    mm_pair(ps, p)
                    if is_direct:
                        nc.vector.reduce_max(
                            out=maxsb[:, B_SH * p : B_SH * (p + 1)],
                            in_=ps.rearrange("p (s j) -> p s j", j=J),
                            axis=mybir.AxisListType.X,
                        )
                    else:
                        if not queue:
                            simcp = treep.tile(
                                [128, TREE_RUN * 2048], bf16, tag="simcp",
                                name=f"simcp_{p}",
                            )
                        else:
                            simcp = queue[0][1]
                            assert queue[-1][0] == p - 1, "tree runs must be contiguous"
                        u = len(queue)
                        nc.scalar.copy(simcp[:, 2048 * u : 2048 * (u + 1)], ps)
                        queue.append((p, simcp))
                        if len(queue) == TREE_RUN:
                            flush_tree(queue)
                    done += 1
                    if done == NORM_AFTER + 1:
                        pano_norms()
                    if done == 16:
                        flush_tree(queue)
                        stage2_half(0)
                flush_tree(queue)
                stage2_half(1)

                nc.sync.dma_start(
                    out=out_d[:].rearrange("(q r) s -> r q s", r=2),
                    in_=out_sb.rearrange("r (q s) -> r q s", s=B_SH),
                )

    nc.finalize()  # Bacc: runs compile() (reg alloc + wait legalization)
    return nc


def _prep_inputs(sat, pano):
    """Host-side shard + layout prep. Returns per-core input maps."""
    import ml_dtypes

    bf16 = ml_dtypes.bfloat16
    pano = np.ascontiguousarray(pano, dtype=np.float32)
    sat = np.ascontiguousarray(sat, dtype=np.float32)

    panoT = np.ascontiguousarray(
        pano.reshape(TOK_PANO, D).T.astype(bf16)
    )  # [128, 4096]
    consts = np.zeros((128, 3), dtype=bf16)
    consts[0:64, 0] = bf16(1.0)
    consts[64:128, 1] = bf16(1.0)
    consts[:, 2] = bf16(1.0)

    in_maps = []
    for c in range(N_CORES):
        satN = np.ascontiguousarray(
            sat[c * B_SH : (c + 1) * B_SH].reshape(TOK_SAT, D)
        )
        in_maps.append({"satN": satN, "panoT": panoT, "consts": consts})
    return in_maps


def kernel(sat_embeddings_unnormalized, pano_embeddings_unnormalized):
    from concourse.bass_utils import run_bass_kernel_spmd

    if "nc" not in _CACHE:
        _CACHE["nc"] = _build_nc()
    nc = _CACHE["nc"]

    in_maps = _prep_inputs(
        np.asarray(sat_embeddings_unnormalized),
        np.asarray(pano_embeddings_unnormalized),
    )
    res = run_bass_kernel_spmd(nc, in_maps, list(range(N_CORES)))
    outs = [np.asarray(res.results[c]["out"], dtype=np.float32) for c in range(N_CORES)]
    return np.concatenate(outs, axis=1)  # [64, 256]
